# revision 1
# baseline (speedup 1.0000x reference)
"""MultiHead InfiniAttention kernel for 8x trn2 NeuronCores.

Sharding: B(2) x H(8) = 16 (batch, head) units, 2 per core (data + head
parallel). Each core computes q/k/v projections for its batch slice and 2
heads, then runs the segment-serial memory scan locally (no collectives).

Layouts on device (per core):
  xT tiles   [128=d-chunk, 512=tok]  via DMA xbar transpose (bf16)
  qT/kT      [128=dk, 512=tok]       (projection in transposed layout)
  v          [tok, dv] natural       (projection with xT slices as lhsT)
  M_aug      [dk, dv+1]  f32 master + bf16 copy; column dv holds z
  scoresT    [128=ktok-chunk, qtok]  so P^T chunks feed PV matmul directly
PV matmul rhs is v augmented with a ones column -> softmax denominator
falls out of the same accumulation.
"""

import os
from contextlib import ExitStack

import ml_dtypes
import numpy as np

import concourse.bass as bass
import concourse.bacc as bacc
import concourse.mybir as mybir
import concourse.tile as tile
from concourse.bass_utils import run_bass_kernel_spmd
from concourse.masks import make_identity

BF16 = mybir.dt.bfloat16
F32 = mybir.dt.float32
ALU = mybir.AluOpType
AF = mybir.ActivationFunctionType

B, S, D = 2, 4096, 1024
H, DK, DV, L = 8, 128, 128, 512
NSEG = S // L
HPC = 2  # heads per core
NCORES = 8
EPS = 1e-6
ISQ = float(1.0 / np.sqrt(DK))

LAST_RESULTS = None  # test harness reads exec_time_ns from here


def _ensure_ntff_hook():
    """Install antenv.axon_hooks shim so trace=True yields NTFF profiles.

    Some agent images lack antenv.axon_hooks; wire the ctypes-based hook
    from trn_agent_boot directly to the axon PJRT .so. Best-effort.
    """
    try:
        from antenv.axon_hooks import get_axon_ntff_profile_hook  # noqa: F401
        return
    except ImportError:
        pass
    try:
        import sys
        import types

        import antenv
        from trn_agent_boot.trn_boot import _ntff_profile_via_ctypes

        mod = types.ModuleType("antenv.axon_hooks")
        _hook = [None]
        mod.set_axon_ntff_profile_hook = lambda h: _hook.__setitem__(0, h)
        mod.get_axon_ntff_profile_hook = lambda: _hook[0]
        sys.modules["antenv.axon_hooks"] = mod
        antenv.axon_hooks = mod
        so_path = "/opt/axon/libaxon_pjrt.so"
        if os.path.exists(so_path):
            mod.set_axon_ntff_profile_hook(_ntff_profile_via_ctypes(so_path))
    except Exception:
        pass


def _build(has_bqk: bool, has_bv: bool):
    nc = bacc.Bacc(None, target_bir_lowering=False, debug=False)
    xb = nc.declare_dram_parameter("xb", [S, D], BF16, isOutput=False)
    wq = nc.declare_dram_parameter("wq", [D, HPC * DK], BF16, isOutput=False)
    wk = nc.declare_dram_parameter("wk", [D, HPC * DK], BF16, isOutput=False)
    wv = nc.declare_dram_parameter("wv", [D, HPC * DV], BF16, isOutput=False)
    beta = nc.declare_dram_parameter("beta", [HPC], F32, isOutput=False)
    bq = bk = bv = None
    if has_bqk:
        bq = nc.declare_dram_parameter("bq", [HPC * DK], F32, isOutput=False)
        bk = nc.declare_dram_parameter("bk", [HPC * DK], F32, isOutput=False)
    if has_bv:
        bv = nc.declare_dram_parameter("bv", [HPC * DV], BF16, isOutput=False)
    out = nc.declare_dram_parameter("out", [S, HPC * DV], F32, isOutput=True)

    with tile.TileContext(nc) as tc, ExitStack() as ctx:
        const = ctx.enter_context(tc.tile_pool(name="const", bufs=1))
        xpool = ctx.enter_context(tc.tile_pool(name="xp", bufs=2))
        qkpool = ctx.enter_context(tc.tile_pool(name="qk", bufs=6))
        elupool = ctx.enter_context(tc.tile_pool(name="elu", bufs=4))
        vpool = ctx.enter_context(tc.tile_pool(name="vp", bufs=6))
        epool = ctx.enter_context(tc.tile_pool(name="ep", bufs=8))
        skpool = ctx.enter_context(tc.tile_pool(name="skp", bufs=3))
        mpool = ctx.enter_context(tc.tile_pool(name="mp", bufs=2))
        spool = ctx.enter_context(tc.tile_pool(name="sp", bufs=8))
        stpool = ctx.enter_context(tc.tile_pool(name="st", bufs=4))
        psA = ctx.enter_context(tc.tile_pool(name="psA", bufs=3, space="PSUM"))
        psS = ctx.enter_context(tc.tile_pool(name="psS", bufs=5, space="PSUM"))
        psQ = psP = psM = psS

        # ---- constants ----
        ident = const.tile([128, 128], BF16, tag="ident", name="ident")
        make_identity(nc, ident[:])
        trilm = const.tile([128, 128], BF16, tag="trilm", name="trilm")
        nc.gpsimd.memset(trilm[:], 1.0)
        nc.gpsimd.affine_select(
            out=trilm[:], in_=trilm[:], pattern=[[1, 128]],
            compare_op=ALU.is_ge, fill=0.0, base=0, channel_multiplier=-1,
        )

        _cc = {}

        def cvec(val):
            if val not in _cc:
                t = const.tile([128, 1], F32, tag=f"c{len(_cc)}", name=f"c{len(_cc)}")
                nc.gpsimd.memset(t[:], val)
                _cc[val] = t
            return _cc[val][:]

        # weights on the scalar HWDGE queue so segment-0's x transpose
        # leads the sync queue (ACT is idle during the preamble)
        wq_t = const.tile([128, 8, HPC * DK], BF16, tag="wq", name="wq")
        nc.scalar.dma_start(wq_t[:], wq.rearrange("(kc p) c -> p kc c", p=128))
        wk_t = const.tile([128, 8, HPC * DK], BF16, tag="wk", name="wk")
        nc.scalar.dma_start(wk_t[:], wk.rearrange("(kc p) c -> p kc c", p=128))
        wv_t = const.tile([128, 8, HPC * DV], BF16, tag="wv", name="wv")
        nc.scalar.dma_start(wv_t[:], wv.rearrange("(kc p) c -> p kc c", p=128))

        bq_t = bk_t = bv_t = None
        if has_bqk:
            bq_t = const.tile([128, HPC], F32, tag="bq", name="bq")
            nc.sync.dma_start(bq_t[:], bq.rearrange("(c p) -> p c", p=128))
            bk_t = const.tile([128, HPC], F32, tag="bk", name="bk")
            nc.sync.dma_start(bk_t[:], bk.rearrange("(c p) -> p c", p=128))
        if has_bv:
            bv_t = const.tile([128, HPC * DV], BF16, tag="bv", name="bv")
            bv_ap = bass.AP(tensor=bv, offset=0, ap=[[0, 128], [1, HPC * DV]])
            nc.gpsimd.dma_start(bv_t[:], bv_ap)

        # gate scalars per head
        betat = const.tile([128, HPC], F32, tag="betat", name="betat")
        beta_ap = bass.AP(tensor=beta, offset=0, ap=[[0, 128], [1, HPC]])
        nc.gpsimd.dma_start(betat[:], beta_ap)
        g_all = const.tile([128, HPC], F32, tag="gall", name="gall")
        nc.scalar.activation(g_all[:], betat[:], AF.Sigmoid)
        ginv, eps_ig, omg_inv = [], [], []
        for h in range(HPC):
            gi = const.tile([128, 1], F32, tag=f"gi{h}", name=f"gi{h}")
            nc.vector.reciprocal(gi[:], g_all[:, h : h + 1])
            ginv.append(gi)
            ei = const.tile([128, 1], F32, tag=f"ei{h}", name=f"ei{h}")
            nc.vector.tensor_scalar_mul(ei[:], gi[:], cvec(EPS))
            eps_ig.append(ei)
            om = const.tile([128, 1], F32, tag=f"om{h}", name=f"om{h}")
            nc.vector.tensor_scalar(om[:], g_all[:, h : h + 1], cvec(-1.0), cvec(1.0), ALU.mult, ALU.add)
            oi = const.tile([128, 1], F32, tag=f"oi{h}", name=f"oi{h}")
            nc.vector.reciprocal(oi[:], om[:])
            omg_inv.append(oi)

        # memory state (M | z) per head: f32 master + bf16 matmul copy
        mf = [mpool.tile([128, DV + 1], F32, tag=f"mf{h}", name=f"mf{h}") for h in range(HPC)]
        mb = [mpool.tile([128, DV + 1], BF16, tag=f"mb{h}", name=f"mb{h}") for h in range(HPC)]
        for h in range(HPC):
            nc.vector.memset(mf[h][:], 0.0)
            nc.gpsimd.memset(mb[h][:], 0.0)

        for s in range(NSEG):
            t0 = s * L
            # ---- load x^T for the whole segment in one xbar transpose:
            # out[p, kc, t] = x[t, kc*128+p]; 1 MiB contiguous DRAM read ----
            xtt = xpool.tile([128, 8, L], BF16, tag="xt", name="xt")
            nc.sync.dma_start(xtt[:, 0:4, :], xb[t0 : t0 + L, 0:512], transpose=True)
            nc.sync.dma_start(xtt[:, 4:8, :], xb[t0 : t0 + L, 512:1024], transpose=True)
            xt = [xtt[:, kc, :] for kc in range(8)]

            # ---- projections ----
            qt, kt = [], []
            for h in range(HPC):
                cs = h * DK
                qps = psA.tile([128, L], F32, tag="big", name="big")
                for kc in range(8):
                    nc.tensor.matmul(
                        qps[:], wq_t[:, kc, cs : cs + DK], xt[kc][:],
                        start=(kc == 0), stop=(kc == 7),
                    )
                qth = qkpool.tile([128, L], BF16, tag=f"qt{h}", name=f"qt{h}")
                if has_bqk:
                    nc.scalar.activation(qth[:], qps[:], AF.Identity, bias=bq_t[:, h : h + 1])
                else:
                    nc.scalar.activation(qth[:], qps[:], AF.Copy)
                qt.append(qth)

                kps = psA.tile([128, L], F32, tag="big", name="big")
                for kc in range(8):
                    nc.tensor.matmul(
                        kps[:], wk_t[:, kc, cs : cs + DK], xt[kc][:],
                        start=(kc == 0), stop=(kc == 7),
                    )
                kth = qkpool.tile([128, L], BF16, tag=f"kt{h}", name=f"kt{h}")
                if has_bqk:
                    nc.scalar.activation(kth[:], kps[:], AF.Identity, bias=bk_t[:, h : h + 1])
                else:
                    nc.vector.tensor_copy(kth[:], kps[:])
                kt.append(kth)

            # v natural: out[tok, col] with xT chunks as stationary operand
            vaug = []
            vps_halves = []
            for half in range(2):
                vps = psA.tile([128, 2, HPC * DV], F32, tag="big", name="big")
                for j2 in range(2):
                    tc4 = half * 2 + j2
                    for kc in range(8):
                        nc.tensor.matmul(
                            vps[:, j2, :],
                            xt[kc][:, tc4 * 128 : (tc4 + 1) * 128],
                            wv_t[:, kc, :],
                            start=(kc == 0), stop=(kc == 7),
                        )
                vps_halves.append(vps)
            for h in range(HPC):
                va = vpool.tile([128, 4, DV + 1], BF16, tag=f"va{h}", name=f"va{h}")
                for half in range(2):
                    nc.scalar.activation(
                        va[:, 2 * half : 2 * half + 2, 0:DV],
                        vps_halves[half][:, :, h * DV : (h + 1) * DV],
                        AF.Copy,
                    )
                nc.gpsimd.memset(va[:, :, DV : DV + 1], 1.0)
                if has_bv:
                    for tc4 in range(4):
                        nc.gpsimd.tensor_add(
                            va[:, tc4, 0:DV], va[:, tc4, 0:DV],
                            bv_t[:, h * DV : (h + 1) * DV],
                        )
                vaug.append(va)

            # ---- elu(x)+1 = relu(x) + exp(min(x,0)) ----
            sqt, skt = [], []
            for h in range(HPC):
                for src, dstlist, tagp in ((qt[h], sqt, "sq"), (kt[h], skt, "sk")):
                    u = elupool.tile([128, L], BF16, tag="u", name="u")
                    nc.vector.tensor_scalar_min(u[:], src[:], cvec(0.0))
                    e = elupool.tile([128, L], BF16, tag="e", name="e")
                    nc.scalar.activation(e[:], u[:], AF.Exp)
                    sx = qkpool.tile([128, L], BF16, tag=f"{tagp}{h}", name=f"{tagp}{h}")
                    nc.vector.scalar_tensor_tensor(
                        sx[:], src[:], cvec(0.0), e[:], op0=ALU.max, op1=ALU.add
                    )
                    dstlist.append(sx)

            # ---- per-head attention + memory step ----
            for h in range(HPC):
                # local causal attention: scoresT chunks -> exp -> mask diag
                expt = []
                for c in range(4):
                    n = L - c * 128
                    scps = psA.tile([128, n], F32, tag="big", name="big")
                    nc.tensor.matmul(
                        scps[:], kt[h][:, c * 128 : (c + 1) * 128], qt[h][:, c * 128 : L],
                        start=True, stop=True,
                    )
                    ex = epool.tile([128, L], BF16, tag="expt", name="expt")
                    nc.scalar.activation(ex[:, c * 128 : L], scps[:], AF.Exp, scale=ISQ)
                    nc.gpsimd.affine_select(
                        out=ex[:, c * 128 : (c + 1) * 128],
                        in_=ex[:, c * 128 : (c + 1) * 128],
                        pattern=[[1, 128]],
                        compare_op=ALU.is_ge,
                        fill=0.0,
                        base=0,
                        channel_multiplier=-1,
                    )
                    expt.append(ex)

                # num_k retrieval (pre-update M), aug column = den.
                # At s=0 the memory is zero: v_delta == vs, skip it all.
                if s > 0:
                    nk = []
                    for half in range(2):
                        ps = psM.tile([128, 2, DV + 1], F32, tag="small", name="nk")
                        for j2 in range(2):
                            j = half * 2 + j2
                            nc.tensor.matmul(
                                ps[:, j2, :], skt[h][:, j * 128 : (j + 1) * 128], mb[h][:],
                                start=True, stop=True,
                            )
                        nk.append(ps)

                    # -1/(den_k + eps) per ktok chunk
                    rk = spool.tile([128, 4, 1], F32, tag="rk", name="rk")
                    rkr = spool.tile([128, 4, 1], F32, tag="rkr", name="rkr")
                    for half in range(2):
                        nc.vector.tensor_scalar(
                            rk[:, 2 * half : 2 * half + 2, :],
                            nk[half][:, :, DV : DV + 1],
                            cvec(-1.0), cvec(-EPS), ALU.mult, ALU.add,
                        )
                    nc.vector.reciprocal(rkr[:], rk[:])

                    # v_delta (aug) = vs - num_k/(den_k+eps), ones column kept
                    vda = vpool.tile([128, 4, DV + 1], BF16, tag="vda", name="vda")
                    for tc4 in range(4):
                        nc.vector.scalar_tensor_tensor(
                            vda[:, tc4, 0:DV],
                            nk[tc4 // 2][:, tc4 % 2, 0:DV],
                            rkr[:, tc4, :],
                            vaug[h][:, tc4, 0:DV],
                            op0=ALU.mult, op1=ALU.add,
                        )
                    nc.gpsimd.memset(vda[:, :, DV : DV + 1], 1.0)
                else:
                    vda = vaug[h]

                # sk natural chunks via PE transpose (packed: one bank, one evict)
                skn = skpool.tile([128, 4, 128], BF16, tag="skn", name="skn")
                trp = psM.tile([128, 4, 128], BF16, tag="small", name="trp")
                for tc4 in range(4):
                    nc.tensor.transpose(
                        trp[:, tc4, :], skt[h][:, tc4 * 128 : (tc4 + 1) * 128], ident[:]
                    )
                nc.vector.tensor_copy(skn[:], trp[:])

                # memory update: M_aug += sk^T @ v_delta_aug
                mps = psM.tile([128, DV + 1], F32, tag="small", name="mps")
                for tc4 in range(4):
                    nc.tensor.matmul(
                        mps[:], skn[:, tc4, :], vda[:, tc4, :],
                        start=(tc4 == 0), stop=(tc4 == 3),
                    )
                mb_old = mb[h]
                mf_new = mpool.tile([128, DV + 1], F32, tag=f"mf{h}", name=f"mf{h}")
                nc.vector.tensor_add(mf_new[:], mf[h][:], mps[:])
                mb_new = mpool.tile([128, DV + 1], BF16, tag=f"mb{h}", name=f"mb{h}")
                nc.scalar.activation(mb_new[:], mf_new[:], AF.Copy)
                mf[h], mb[h] = mf_new, mb_new

                # PV (+denominator): triangular accumulation over ktok chunks
                pv = []
                for half in range(2):
                    ps = psP.tile([128, 2, DV + 1], F32, tag="small", name="pv")
                    for j2 in range(2):
                        j = half * 2 + j2
                        for c in range(j + 1):
                            nc.tensor.matmul(
                                ps[:, j2, :],
                                expt[c][:, j * 128 : (j + 1) * 128],
                                vaug[h][:, c, :],
                                start=(c == 0), stop=(c == j),
                            )
                    pv.append(ps)

                # memory retrieval for q (pre-update M), emitted late to
                # shorten the PSUM lifetime; reads the old mb tile.
                if s > 0:
                    nq = []
                    for half in range(2):
                        ps = psQ.tile([128, 2, DV + 1], F32, tag="small", name="nq")
                        for j2 in range(2):
                            j = half * 2 + j2
                            nc.tensor.matmul(
                                ps[:, j2, :], sqt[h][:, j * 128 : (j + 1) * 128], mb_old[:],
                                start=True, stop=True,
                            )
                        nq.append(ps)

                # reciprocals: cols 0..3 = g/(den_q+eps), 4..7 = (1-g)/den_s
                rall = spool.tile([128, 8, 1], F32, tag="rall", name="rall")
                rallr = spool.tile([128, 8, 1], F32, tag="rallr", name="rallr")
                for half in range(2):
                    if s > 0:
                        nc.vector.tensor_scalar(
                            rall[:, 2 * half : 2 * half + 2, :],
                            nq[half][:, :, DV : DV + 1],
                            ginv[h][:], eps_ig[h][:], ALU.mult, ALU.add,
                        )
                    nc.vector.tensor_scalar(
                        rall[:, 4 + 2 * half : 6 + 2 * half, :],
                        pv[half][:, :, DV : DV + 1],
                        omg_inv[h][:], None, ALU.mult,
                    )
                nc.vector.reciprocal(rallr[:], rall[:])

                # a = c1*num_q + c2*pv  -> staging -> DRAM
                t2 = stpool.tile([128, 4, DV], F32, tag="t2", name="t2")
                staging = stpool.tile([128, 4, DV], F32, tag="stg", name="stg")
                for j in range(4):
                    if s > 0:
                        nc.scalar.activation(
                            t2[:, j, :], pv[j // 2][:, j % 2, 0:DV], AF.Copy,
                            scale=rallr[:, 4 + j, :],
                        )
                        nc.vector.scalar_tensor_tensor(
                            staging[:, j, :],
                            nq[j // 2][:, j % 2, 0:DV],
                            rallr[:, j, :],
                            t2[:, j, :],
                            op0=ALU.mult, op1=ALU.add,
                        )
                    else:
                        nc.scalar.activation(
                            staging[:, j, :], pv[j // 2][:, j % 2, 0:DV], AF.Copy,
                            scale=rallr[:, 4 + j, :],
                        )
                outv = out[t0 : t0 + L, h * DV : (h + 1) * DV].rearrange(
                    "(j p) e -> p j e", p=128
                )
                nc.sync.dma_start(outv, staging[:])

    nc.finalize()
    return nc


_CACHE = {}


def _get_program(has_bqk, has_bv):
    key = (has_bqk, has_bv)
    if key not in _CACHE:
        _CACHE[key] = _build(*key)
    return _CACHE[key]


def kernel(x, Wq, bq, Wk, bk, Wv, bv, beta, **_unused):
    global LAST_RESULTS
    x = np.asarray(x, dtype=np.float32)
    Wq = np.asarray(Wq, dtype=np.float32)
    Wk = np.asarray(Wk, dtype=np.float32)
    Wv = np.asarray(Wv, dtype=np.float32)
    bq = np.asarray(bq, dtype=np.float32)
    bk = np.asarray(bk, dtype=np.float32)
    bv = np.asarray(bv, dtype=np.float32)
    beta = np.asarray(beta, dtype=np.float32)

    has_bqk = bool(np.any(bq) or np.any(bk))
    has_bv = bool(np.any(bv))
    nc = _get_program(has_bqk, has_bv)

    bf = ml_dtypes.bfloat16
    in_maps = []
    for c in range(NCORES):
        b = c // 4
        h0 = HPC * (c % 4)
        cols = slice(h0 * DK, (h0 + HPC) * DK)
        m = {
            "xb": np.ascontiguousarray(x[b]).astype(bf),
            "wq": np.ascontiguousarray(Wq[:, cols]).astype(bf),
            "wk": np.ascontiguousarray(Wk[:, cols]).astype(bf),
            "wv": np.ascontiguousarray(Wv[:, cols]).astype(bf),
            "beta": np.ascontiguousarray(beta[h0 : h0 + HPC]),
        }
        if has_bqk:
            m["bq"] = np.ascontiguousarray(bq[cols])
            m["bk"] = np.ascontiguousarray(bk[cols])
        if has_bv:
            m["bv"] = np.ascontiguousarray(bv[cols]).astype(bf)
        in_maps.append(m)

    trace = os.environ.get("BASS_KERNEL_TRACE", "0") == "1"
    if trace:
        _ensure_ntff_hook()
    res = run_bass_kernel_spmd(nc, in_maps, core_ids=list(range(NCORES)), trace=trace)
    LAST_RESULTS = res

    outp = np.empty((B, S, H * DV), np.float32)
    for c in range(NCORES):
        b = c // 4
        h0 = HPC * (c % 4)
        outp[b, :, h0 * DV : (h0 + HPC) * DV] = res.results[c]["out"]
    return outp

